# revision 9
# baseline (speedup 1.0000x reference)
"""Trainium2 Bass kernel: 4-layer GPT-2-style transformer (B=2, S=2048, D=1024,
H=16, DH=64, M=4096, V=50257) on 8 NeuronCores.

Sharding (one SPMD program, no core-dependent control flow or addressing):
  - Residual stream / LN / MLP / W_O: sequence-parallel. Core c owns batch
    b = c//4, tokens [512*g, 512*(g+1)) with g = c%4.
  - Attention: head-parallel. Core c computes heads {2c, 2c+1} for BOTH
    batches and all tokens. Per layer: one 8-core AllGather of x_ln^T,
    then a fused qkv+attention pipeline over eight 512-token chunks
    (batch-interleaved for ILP), then one 8-core AllToAll routing z back
    from head-shards to token-shards.
  - Unembed: vocab-parallel. 8-core AllGather of final^T, then every core
    computes all 4096 tokens x its 6283-column vocab shard.
  - All matmul operands are float16 (weights converted host-side); PSUM
    accumulation is f32. The residual stream stays f32r on-chip.
"""

import sys, os
sys.path.insert(0, '/opt/trn_rl_repo')
os.environ.setdefault('MYCRO_LOCAL_CACHE', '1')

from contextlib import ExitStack

import numpy as np

import concourse.bass as bass
import concourse.bacc as bacc
import concourse.mybir as mybir
import concourse.tile as tile
from concourse.bass_utils import run_bass_kernel_spmd
from concourse.masks import make_identity

# model dims
B, S, V, D, H, DH, MLPD, L = 2, 2048, 50257, 1024, 16, 64, 4096, 4
EPS = 1e-5
NCORES = 8
G = 4                 # sequence-parallel degree within a batch
T = S // G            # 512 local tokens per core
BS = B * S            # 4096 total tokens
DT = D // 128         # 8 d-tiles
INV_SQRT_DH = float(1.0 / np.sqrt(DH))
VSH = (V + NCORES - 1) // NCORES      # 6283 vocab shard width
NV = (VSH + 511) // 512               # 13 vocab n-tiles (last is 139 wide)

F32 = mybir.dt.float32
F32R = mybir.dt.float32r
I32 = mybir.dt.int32
F16 = mybir.dt.float16
AF = mybir.ActivationFunctionType
OP = mybir.AluOpType

ALL8 = [[0, 1, 2, 3, 4, 5, 6, 7]]

# chunk processing order: (b2, g) pairs, batch-interleaved so the two
# batches' attention chains overlap
CHUNKS = [(0, 0), (1, 0), (0, 1), (1, 1), (0, 2), (1, 2), (0, 3), (1, 3)]

_COMPILED = None


def ts(i, n):
    return slice(i * n, (i + 1) * n)


def _build():
    nc = bacc.Bacc("TRN2", target_bir_lowering=False, debug=False,
                   num_devices=NCORES)

    # ---------------- I/O -----------------
    tok_d = nc.dram_tensor("tok", [T], I32, kind="ExternalInput")
    we_d = nc.dram_tensor("we", [V, D], F32, kind="ExternalInput")
    wpos_d = nc.dram_tensor("wposT", [D, T], F32, kind="ExternalInput")
    # per-core head slice: q-pair (128) | k-pair (128)
    wqk_d = nc.dram_tensor("wqk", [L, D, 256], F16, kind="ExternalInput")
    wv_d = nc.dram_tensor("wv", [L, D, 128], F16, kind="ExternalInput")
    wo_d = nc.dram_tensor("wo", [L, H * DH, D], F16, kind="ExternalInput")
    wi_d = nc.dram_tensor("wi", [L, D, MLPD], F16, kind="ExternalInput")
    wout_d = nc.dram_tensor("wout", [L, MLPD, D], F16, kind="ExternalInput")
    wu_d = nc.dram_tensor("wu", [D, VSH], F16, kind="ExternalInput")
    logits_d = nc.dram_tensor("logits", [BS, VSH], F16, kind="ExternalOutput")

    # ------------- collective buffers -------------
    xb = nc.dram_tensor("xb", [D, T], F16)               # x_ln bounce
    xg = nc.dram_tensor("xg", [NCORES * D, T], F16, addr_space="Shared")
    zb = nc.dram_tensor("zb", [NCORES * 128, T], F16)    # z bounce (A2A in)
    zg = nc.dram_tensor("zg", [H * DH, T], F16)          # A2A out
    fb = nc.dram_tensor("fb", [D, T], F16)               # final bounce
    fg = nc.dram_tensor("fg", [NCORES * D, T], F16, addr_space="Shared")

    with tile.TileContext(nc) as tc:
        # PSUM pools live for the whole kernel (8 banks total).
        with tc.tile_pool(name="ps_mm", bufs=2, space="PSUM") as pps_mm, \
             tc.tile_pool(name="ps_sc", bufs=2, space="PSUM") as pps_sc, \
             tc.tile_pool(name="ps_z", bufs=2, space="PSUM") as pps_z, \
             tc.tile_pool(name="ps_vec", bufs=2, space="PSUM") as pps_vec:

            with ExitStack() as lctx:
                p1 = lctx.enter_context(tc.tile_pool(name="const", bufs=1))
                presid = lctx.enter_context(tc.tile_pool(name="presid",
                                                         bufs=8))
                pxln = lctx.enter_context(tc.tile_pool(name="pxln", bufs=8))
                pbig = lctx.enter_context(tc.tile_pool(name="pbig", bufs=3))
                pvaug = lctx.enter_context(tc.tile_pool(name="pvaug",
                                                        bufs=8))
                pxj = lctx.enter_context(tc.tile_pool(name="pxj", bufs=3))
                pvt = lctx.enter_context(tc.tile_pool(name="pvt", bufs=2))
                pex = lctx.enter_context(tc.tile_pool(name="pex", bufs=5))
                ppost = lctx.enter_context(tc.tile_pool(name="ppost",
                                                        bufs=18))
                pw = lctx.enter_context(tc.tile_pool(name="pw", bufs=4))
                pqw = lctx.enter_context(tc.tile_pool(name="pqw", bufs=3))
                psq = lctx.enter_context(tc.tile_pool(name="psq", bufs=2))
                ptmp = lctx.enter_context(tc.tile_pool(name="ptmp", bufs=2))
                pln = lctx.enter_context(tc.tile_pool(name="pln", bufs=8))
                prc = lctx.enter_context(tc.tile_pool(name="prc", bufs=2))

                # ---------- constants ----------
                ident = p1.tile([128, 128], F32, tag="ident")
                make_identity(nc, ident[:])
                identr = p1.tile([128, 128], F32R, tag="identr")
                nc.vector.tensor_copy(identr[:], ident[:])
                onesf = p1.tile([128, 128], F32, tag="onesf")
                nc.vector.memset(onesf[:], 1.0)
                ones16 = p1.tile([128, 8], F16, tag="ones16")
                nc.vector.memset(ones16[:], 1.0)
                ones_c = p1.tile([128, 1], F32R, tag="ones_c")
                nc.vector.tensor_copy(ones_c[:], onesf[:, 0:1])
                ones_r64 = p1.tile([1, 64], F32, tag="ones_r64")
                nc.vector.tensor_copy(ones_r64[:], onesf[0:1, 0:64])
                ones_r128 = p1.tile([1, 128], F32R, tag="ones_r128")
                nc.vector.tensor_copy(ones_r128[:], onesf[0:1, :])
                eps_t = p1.tile([1, 1], F32, tag="eps")
                nc.vector.memset(eps_t[:], EPS)
                # multiplicative causal masks for the 4 key tiles of a
                # diagonal 512x512 chunk; mask[k, q] = 1 iff q >= k + off
                masks = []
                for mi in range(4):
                    off = 128 * mi
                    mk = p1.tile([128, T], F16, tag=f"mask{mi}")
                    nc.gpsimd.memset(mk[:], 1.0)
                    nc.gpsimd.affine_select(
                        out=mk[:], in_=mk[:], compare_op=OP.is_ge,
                        fill=0.0, base=-off, pattern=[[1, T]],
                        channel_multiplier=-1)
                    masks.append(mk)

                # residual stream x^T, [D on partitions, T tokens], f32r
                resid = [presid.tile([128, T], F32R, tag="resid",
                                      name=f"resid{i}")
                         for i in range(DT)]

                def layer_norm(src_tiles, dst_tiles):
                    """dst = (src - mean_D) / sqrt(var_D + eps) per token;
                    x^T layout, stats over the partition (D) axis via
                    ones-matmuls.  rstd = exp(-0.5*ln(var+eps)) keeps the
                    ACT engine on the exp/ln table set (no sqrt set)."""
                    sum_ps = pps_vec.tile([1, T], F32, tag="vec")
                    sq_ps = pps_vec.tile([1, T], F32, tag="vec")
                    for d in range(DT):
                        sq = psq.tile([128, T], F32R, tag="sq")
                        nc.scalar.activation(sq[:], src_tiles[d][:],
                                             AF.Square)
                        nc.tensor.matmul(sum_ps[:], ones_c[:],
                                         src_tiles[d][:],
                                         start=(d == 0), stop=(d == DT - 1))
                        nc.tensor.matmul(sq_ps[:], ones_c[:], sq[:],
                                         start=(d == 0), stop=(d == DT - 1))
                    mean = pln.tile([1, T], F32R, tag="ln")
                    nc.scalar.mul(mean[:], sum_ps[:], 1.0 / D)
                    ems = pln.tile([1, T], F32, tag="ln")
                    nc.scalar.mul(ems[:], sq_ps[:], 1.0 / D)
                    m2 = pln.tile([1, T], F32, tag="ln")
                    nc.scalar.activation(m2[:], mean[:], AF.Square)
                    var = pln.tile([1, T], F32, tag="ln")
                    nc.vector.tensor_tensor(out=var[:], in0=ems[:],
                                            in1=m2[:], op=OP.subtract)
                    lnv = pln.tile([1, T], F32, tag="ln")
                    nc.scalar.activation(lnv[:], var[:], AF.Ln,
                                         bias=eps_t[:])
                    rstd = pln.tile([1, T], F32R, tag="ln")
                    nc.scalar.activation(rstd[:], lnv[:], AF.Exp,
                                         scale=-0.5)
                    bc_m = pps_vec.tile([128, T], F32, tag="vec")
                    nc.tensor.matmul(bc_m[:], ones_r128[:], mean[:],
                                     start=True, stop=True)
                    bc_r = pps_vec.tile([128, T], F32, tag="vec")
                    nc.tensor.matmul(bc_r[:], ones_r128[:], rstd[:],
                                     start=True, stop=True)
                    for d in range(DT):
                        tmp = ptmp.tile([128, T], F32, tag="lntmp")
                        nc.vector.tensor_tensor(out=tmp[:],
                                                in0=src_tiles[d][:],
                                                in1=bc_m[:], op=OP.subtract)
                        nc.vector.tensor_tensor(out=dst_tiles[d][:],
                                                in0=tmp[:], in1=bc_r[:],
                                                op=OP.mult)

                # ================= embedding =================
                with nc.named_scope("embed"), \
                     tc.tile_pool(name="pemb", bufs=8) as pemb, \
                     tc.tile_pool(name="pidx", bufs=2) as pidx:
                    wpos_sb = []
                    for d in range(DT):
                        wp = pemb.tile([128, T], F32, tag="wpos")
                        nc.sync.dma_start(wp[:], wpos_d[ts(d, 128), :])
                        wpos_sb.append(wp)
                    for t in range(T // 128):
                        it = pidx.tile([128, 1], I32, tag="idx")
                        nc.sync.dma_start(
                            it[:],
                            tok_d[ts(t, 128)].rearrange("(p o) -> p o", o=1))
                        xe = pidx.tile([128, D], F32, tag="xe")
                        nc.gpsimd.indirect_dma_start(
                            out=xe[:], out_offset=None, in_=we_d[:],
                            in_offset=bass.IndirectOffsetOnAxis(
                                ap=it[:, :1], axis=0))
                        for d in range(DT):
                            tp = pps_vec.tile([128, 128], F32, tag="vec")
                            nc.tensor.transpose(tp[:], xe[:, ts(d, 128)],
                                                ident[:])
                            nc.vector.tensor_tensor(
                                out=resid[d][:, ts(t, 128)], in0=tp[:],
                                in1=wpos_sb[d][:, ts(t, 128)], op=OP.add)

                # ================= layers =================
                for l in range(L):
                    # ---- LN1 + 8-core AllGather of x_ln^T ----
                    with nc.named_scope(f"l{l}_ln1"):
                        xln = [pxln.tile([128, T], F16, tag="xln",
                                         name=f"xln_{l}_{i}")
                               for i in range(DT)]
                        layer_norm(resid, xln)
                        for d in range(DT):
                            nc.sync.dma_start(xb[ts(d, 128), :], xln[d][:])
                        nc.gpsimd.collective_compute(
                            "AllGather", OP.bypass, replica_groups=ALL8,
                            ins=[xb[:]], outs=[xg[:]])

                    # ---- fused qkv + attention over 512-token chunks ----
                    with nc.named_scope(f"l{l}_attn"):
                        # per-core q/k in head-major layout
                        # [128 = 2 heads x 64dh, 4096 tokens]
                        qhp = pbig.tile([128, BS], F16, tag="big")
                        khp = pbig.tile([128, BS], F16, tag="big")
                        zT = pbig.tile([128, BS], F16, tag="big")
                        # v (normal layout) + ones column, per (head, batch):
                        # [128 tok, 8 keytiles, 65]
                        vaug = [[[pvaug.tile([128, 8, 65], F16, tag="vaug",
                                             name=f"vaug{l}_{h}_{b2}_{j}")
                                  for j in range(2)] for b2 in range(2)]
                                for h in range(2)]
                        for h in range(2):
                            for b2 in range(2):
                                for j in range(2):
                                    nc.vector.tensor_copy(
                                        vaug[h][b2][j][:, :, 64:65],
                                        ones16[:, :].rearrange(
                                            "p (a b) -> p a b", b=1))
                        wq = pqw.tile([128, 8, 128], F16, tag="qw")
                        nc.sync.dma_start(
                            wq[:],
                            wqk_d[l, :, 0:128].rearrange(
                                "(k p) c -> p k c", p=128))
                        wk = pqw.tile([128, 8, 128], F16, tag="qw")
                        nc.sync.dma_start(
                            wk[:],
                            wqk_d[l, :, 128:256].rearrange(
                                "(k p) c -> p k c", p=128))
                        wv = pqw.tile([128, 8, 128], F16, tag="qw")
                        nc.sync.dma_start(
                            wv[:],
                            wv_d[l].rearrange("(k p) c -> p k c", p=128))

                        for (b2, g) in CHUNKS:
                            c8 = 4 * b2 + g          # owning core index
                            col = slice(2048 * b2 + T * g,
                                        2048 * b2 + T * (g + 1))
                            xj = pxj.tile([128, 8, T], F16, tag="xj")
                            nc.sync.dma_start(
                                xj[:],
                                xg[ts(c8, D), :].rearrange(
                                    "(k p) c -> p k c", p=128))
                            # q / k for my 2 heads, this chunk
                            for w, dst in ((wq, qhp), (wk, khp)):
                                ps = pps_mm.tile([128, T], F32, tag="mm")
                                for k in range(DT):
                                    nc.tensor.matmul(
                                        ps[:], w[:, k, :], xj[:, k, :],
                                        start=(k == 0), stop=(k == DT - 1))
                                nc.any.tensor_copy(dst[:, col], ps[:])
                            # v^T then PE-transpose to normal layout
                            psv = pps_mm.tile([128, T], F32, tag="mm")
                            for k in range(DT):
                                nc.tensor.matmul(
                                    psv[:], wv[:, k, :], xj[:, k, :],
                                    start=(k == 0), stop=(k == DT - 1))
                            vt = pvt.tile([128, T], F32R, tag="vt")
                            nc.any.tensor_copy(vt[:], psv[:])
                            for tt in range(4):
                                kt = 4 * g + tt      # key tile in batch b2
                                tp = pps_vec.tile([128, 128], F32R,
                                                  tag="vec")
                                nc.tensor.transpose(
                                    tp[:], vt[:, ts(tt, 128)], identr[:])
                                for h in range(2):
                                    nc.any.tensor_copy(
                                        vaug[h][b2][kt // 8][:, kt % 8,
                                                             0:64],
                                        tp[:, ts(h, 64)])
                            # ---- attention for this chunk's 512 queries
                            zps = [pps_z.tile([65, T], F32, tag="z",
                                              name=f"zps{hh}")
                                   for hh in range(2)]
                            nk = 4 * g + 4
                            for kt in range(nk):
                                for hh in range(2):
                                    scps = pps_sc.tile([128, T], F32,
                                                       tag="sc")
                                    nc.tensor.matmul(
                                        scps[:],
                                        khp[ts(hh, 64),
                                            2048 * b2 + 128 * kt:
                                            2048 * b2 + 128 * (kt + 1)],
                                        qhp[ts(hh, 64), col],
                                        start=True, stop=True)
                                    ex = pex.tile([128, T], F16, tag="ex")
                                    nc.scalar.activation(
                                        ex[:], scps[:], AF.Exp,
                                        scale=INV_SQRT_DH)
                                    if kt >= 4 * g:     # diagonal chunk
                                        nc.vector.tensor_tensor(
                                            out=ex[:], in0=ex[:],
                                            in1=masks[kt - 4 * g][:],
                                            op=OP.mult)
                                    nc.tensor.matmul(
                                        zps[hh][:],
                                        vaug[hh][b2][kt // 8][:, kt % 8,
                                                              0:65],
                                        ex[:], start=(kt == 0),
                                        stop=(kt == nk - 1))
                            for hh in range(2):
                                dn = prc.tile([1, T], F32, tag="dn")
                                nc.any.tensor_copy(dn[:], zps[hh][64:65, :])
                                rc = prc.tile([1, T], F32, tag="rc")
                                nc.vector.reciprocal_approx_fast(
                                    out=rc[:], in_=dn[:])
                                bc = pps_vec.tile([64, T], F32, tag="vec")
                                nc.tensor.matmul(
                                    bc[:], ones_r64[:], rc[:],
                                    start=True, stop=True)
                                bcs = prc.tile([64, T], F32, tag="bcs")
                                nc.any.tensor_copy(bcs[:], bc[:])
                                nc.vector.tensor_tensor(
                                    out=zT[ts(hh, 64), col],
                                    in0=zps[hh][0:64, :], in1=bcs[:],
                                    op=OP.mult)
                            # route this chunk's z to its owning core
                            nc.sync.dma_start(zb[ts(c8, 128), :],
                                              zT[:, col])

                    # ---- z AllToAll (head-shard -> token-shard) + W_O ----
                    with nc.named_scope(f"l{l}_wo"):
                        nc.gpsimd.collective_compute(
                            "AllToAll", OP.bypass, replica_groups=ALL8,
                            ins=[zb[:]], outs=[zg[:]])
                        zgt = pxj.tile([128, 8, T], F16, tag="xj")
                        nc.sync.dma_start(
                            zgt[:],
                            zg[:, :].rearrange("(k p) c -> p k c", p=128))
                        for m in range(DT):
                            w = pw.tile([128, 8, 128], F16, tag="w")
                            nc.sync.dma_start(
                                w[:],
                                wo_d[l, :, ts(m, 128)].rearrange(
                                    "(k p) c -> p k c", p=128))
                            ps = pps_mm.tile([128, T], F32, tag="mm")
                            for k in range(DT):
                                nc.tensor.matmul(
                                    ps[:], w[:, k, :], zgt[:, k, :],
                                    start=(k == 0), stop=(k == DT - 1))
                            nc.vector.tensor_tensor(out=resid[m][:],
                                                    in0=resid[m][:],
                                                    in1=ps[:], op=OP.add)

                    # ---- LN2 + MLP ----
                    with nc.named_scope(f"l{l}_mlp"):
                        xln2 = [pxln.tile([128, T], F16, tag="xln",
                                          name=f"xln2_{l}_{i}")
                                for i in range(DT)]
                        layer_norm(resid, xln2)
                        for half in range(2):
                            post = []
                            for mh in range(16):
                                m = 16 * half + mh
                                w = pw.tile([128, 8, 128], F16, tag="w")
                                nc.sync.dma_start(
                                    w[:],
                                    wi_d[l, :, ts(m, 128)].rearrange(
                                        "(k p) c -> p k c", p=128))
                                ps = pps_mm.tile([128, T], F32, tag="mm")
                                for k in range(DT):
                                    nc.tensor.matmul(
                                        ps[:], w[:, k, :], xln2[k][:],
                                        start=(k == 0), stop=(k == DT - 1))
                                po = ppost.tile([128, T], F16, tag="post")
                                nc.scalar.activation(po[:], ps[:],
                                                     AF.Gelu_apprx_tanh)
                                post.append(po)
                            for m in range(DT):
                                w = pw.tile([128, 16, 128], F16, tag="w")
                                nc.sync.dma_start(
                                    w[:],
                                    wout_d[l, ts(half, 2048),
                                           ts(m, 128)].rearrange(
                                        "(k p) c -> p k c", p=128))
                                ps = pps_mm.tile([128, T], F32, tag="mm")
                                for k in range(16):
                                    nc.tensor.matmul(
                                        ps[:], w[:, k, :], post[k][:],
                                        start=(k == 0), stop=(k == 15))
                                nc.vector.tensor_tensor(out=resid[m][:],
                                                        in0=resid[m][:],
                                                        in1=ps[:],
                                                        op=OP.add)

                # ---- final LN + 8-core gather ----
                with nc.named_scope("final_ln"):
                    xf = [pxln.tile([128, T], F16, tag="xln", name=f"xf{i}")
                          for i in range(DT)]
                    layer_norm(resid, xf)
                    for d in range(DT):
                        nc.sync.dma_start(fb[ts(d, 128), :], xf[d][:])
                    nc.gpsimd.collective_compute(
                        "AllGather", OP.bypass, replica_groups=ALL8,
                        ins=[fb[:]], outs=[fg[:]])

            # ================= unembed (vocab shard) =================
            with nc.named_scope("unembed"), \
                 tc.tile_pool(name="uf", bufs=32) as puf, \
                 tc.tile_pool(name="uw", bufs=2) as puw, \
                 tc.tile_pool(name="uo", bufs=3) as puo:
                ft = []
                for blk in range(NCORES):
                    for t in range(T // 128):
                        f = puf.tile([128, 8, 128], F16, tag="ft")
                        nc.sync.dma_start(
                            f[:],
                            fg[ts(blk, D), ts(t, 128)].rearrange(
                                "(k p) c -> p k c", p=128))
                        ft.append(f)
                for n in range(NV):
                    wn = min(512, VSH - 512 * n)      # 512 or 139
                    w = puw.tile([128, 8, 512], F16, tag="wu")
                    nc.sync.dma_start(
                        w[:, :, :wn],
                        wu_d[:, 512 * n: 512 * n + wn].rearrange(
                            "(k p) c -> p k c", p=128))
                    for tt in range(BS // 128):
                        if tt % 4 == 0:
                            o = puo.tile([128, 4, 512], F16, tag="lgo")
                        ps = pps_mm.tile([128, 512], F32, tag="mm")
                        for k in range(DT):
                            nc.tensor.matmul(ps[:, :wn], ft[tt][:, k, :],
                                             w[:, k, :wn],
                                             start=(k == 0),
                                             stop=(k == DT - 1))
                        nc.any.tensor_copy(o[:, tt % 4, :wn], ps[:, :wn])
                        if tt % 4 == 3:
                            nc.sync.dma_start(
                                logits_d[ts(tt // 4, 512),
                                         512 * n: 512 * n + wn].rearrange(
                                    "(a p) c -> p a c", p=128),
                                o[:, :, :wn])

    nc.compile()
    return nc


def _prep_inputs(inputs):
    """Validate + build the 8 per-core input maps (host-side sharding)."""
    inp = {k: np.asarray(v) for k, v in inputs.items()}

    for name in ('b_Q', 'b_K', 'b_V', 'b_O', 'b_in', 'b_out', 'b_U',
                 'ln1_b', 'ln2_b', 'lnf_b'):
        if inp[name].any():
            raise NotImplementedError(f"nonzero {name} not supported")
    for name in ('ln1_w', 'ln2_w', 'lnf_w'):
        if not np.all(inp[name] == 1.0):
            raise NotImplementedError(f"non-unit {name} not supported")

    tokens = inp['tokens'].astype(np.int32)                      # [B, S]
    W_E = np.ascontiguousarray(inp['W_E'], np.float32)           # [V, D]
    W_posT = np.ascontiguousarray(inp['W_pos'].T, np.float32)    # [D, S]
    WQ, WK, WV = inp['W_Q'], inp['W_K'], inp['W_V']              # [L,H,D,DH]
    WO = np.ascontiguousarray(
        inp['W_O'].reshape(L, H * DH, D), np.float16)            # [L,HDH,D]
    WI = np.ascontiguousarray(inp['W_in'], np.float16)           # [L,D,M]
    WOUT = np.ascontiguousarray(inp['W_out'], np.float16)        # [L,M,D]
    WU = inp['W_U'].astype(np.float32)                           # [D, V]

    in_maps = []
    for c in range(NCORES):
        b, g = c // G, c % G
        hsel = slice(2 * c, 2 * c + 2)
        # [L, 2, D, DH] -> [L, D, 2*DH]
        wq_c = WQ[:, hsel].transpose(0, 2, 1, 3).reshape(L, D, 2 * DH)
        wk_c = WK[:, hsel].transpose(0, 2, 1, 3).reshape(L, D, 2 * DH)
        wqk_c = np.ascontiguousarray(
            np.concatenate([wq_c, wk_c], axis=2), np.float16)
        wv_c = np.ascontiguousarray(
            WV[:, hsel].transpose(0, 2, 1, 3).reshape(L, D, 2 * DH),
            np.float16)
        lo, hi = VSH * c, min(VSH * (c + 1), V)
        wu_c = np.zeros((D, VSH), np.float16)
        wu_c[:, :hi - lo] = WU[:, lo:hi]
        in_maps.append({
            'tok': np.ascontiguousarray(tokens[b, ts(g, T)]),
            'we': W_E,
            'wposT': np.ascontiguousarray(W_posT[:, ts(g, T)]),
            'wqk': wqk_c,
            'wv': wv_c,
            'wo': WO,
            'wi': WI,
            'wout': WOUT,
            'wu': wu_c,
        })
    return in_maps


def kernel(**inputs):
    global _COMPILED
    if _COMPILED is None:
        _COMPILED = _build()
    nc = _COMPILED

    in_maps = _prep_inputs(inputs)
    trace = bool(int(os.environ.get('KERNEL_TRACE', '0')))
    res = run_bass_kernel_spmd(nc, in_maps, core_ids=list(range(NCORES)),
                               trace=trace)
    kernel.last_results = res

    logits = np.empty((B, S, V), np.float32)
    for c in range(NCORES):
        lg = res.results[c]['logits']                 # [4096, VSH] f16
        lo = VSH * c
        hi = min(VSH * (c + 1), V)
        if hi <= lo:
            continue
        for blk in range(NCORES):
            bb, gg = blk // G, blk % G
            logits[bb, ts(gg, T), lo:hi] = \
                lg[ts(blk, T), :hi - lo].astype(np.float32)
    return logits


# revision 11
# speedup vs baseline: 1.0609x; 1.0609x over previous
"""Trainium2 Bass kernel: 4-layer GPT-2-style transformer (B=2, S=2048, D=1024,
H=16, DH=64, M=4096, V=50257) on 8 NeuronCores.

Sharding (one SPMD program, no core-dependent control flow or addressing):
  - Residual stream / LN / MLP / W_O: sequence-parallel. Core c owns batch
    b = c//4, tokens [512*g, 512*(g+1)) with g = c%4.
  - Attention: head-parallel. Core c computes heads {2c, 2c+1} for BOTH
    batches and all tokens. Per layer: one 8-core AllGather of x_ln^T,
    then a fused qkv+attention pipeline over eight 512-token chunks
    (batch-interleaved for ILP), then one 8-core AllToAll routing z back
    from head-shards to token-shards.
  - Unembed: vocab-parallel. 8-core AllGather of final^T, then every core
    computes all 4096 tokens x its 6283-column vocab shard.
  - All matmul operands are float16 (weights converted host-side); PSUM
    accumulation is f32. The residual stream stays f32r on-chip.
"""

import sys, os
sys.path.insert(0, '/opt/trn_rl_repo')
os.environ.setdefault('MYCRO_LOCAL_CACHE', '1')

from contextlib import ExitStack

import numpy as np

import concourse.bass as bass
import concourse.bacc as bacc
import concourse.mybir as mybir
import concourse.tile as tile
from concourse.bass_utils import run_bass_kernel_spmd
from concourse.masks import make_identity

# model dims
B, S, V, D, H, DH, MLPD, L = 2, 2048, 50257, 1024, 16, 64, 4096, 4
EPS = 1e-5
NCORES = 8
G = 4                 # sequence-parallel degree within a batch
T = S // G            # 512 local tokens per core
BS = B * S            # 4096 total tokens
DT = D // 128         # 8 d-tiles
INV_SQRT_DH = float(1.0 / np.sqrt(DH))
VSH = (V + NCORES - 1) // NCORES      # 6283 vocab shard width
NV = (VSH + 511) // 512               # 13 vocab n-tiles (last is 139 wide)

F32 = mybir.dt.float32
F32R = mybir.dt.float32r
I32 = mybir.dt.int32
F16 = mybir.dt.float16
AF = mybir.ActivationFunctionType
OP = mybir.AluOpType

ALL8 = [[0, 1, 2, 3, 4, 5, 6, 7]]

# chunk processing order: (b2, g) pairs, batch-interleaved so the two
# batches' attention chains overlap
CHUNKS = [(0, 0), (1, 0), (0, 1), (1, 1), (0, 2), (1, 2), (0, 3), (1, 3)]

_COMPILED = None


def ts(i, n):
    return slice(i * n, (i + 1) * n)


def _build():
    nc = bacc.Bacc("TRN2", target_bir_lowering=False, debug=False,
                   num_devices=NCORES)

    # ---------------- I/O -----------------
    tok_d = nc.dram_tensor("tok", [T], I32, kind="ExternalInput")
    we_d = nc.dram_tensor("we", [V, D], F32, kind="ExternalInput")
    wpos_d = nc.dram_tensor("wposT", [D, T], F32, kind="ExternalInput")
    # per-core head slice: q-pair (128) | k-pair (128)
    wqk_d = nc.dram_tensor("wqk", [L, D, 256], F16, kind="ExternalInput")
    wv_d = nc.dram_tensor("wv", [L, D, 128], F16, kind="ExternalInput")
    wo_d = nc.dram_tensor("wo", [L, H * DH, D], F16, kind="ExternalInput")
    wi_d = nc.dram_tensor("wi", [L, D, MLPD], F16, kind="ExternalInput")
    wout_d = nc.dram_tensor("wout", [L, MLPD, D], F16, kind="ExternalInput")
    wu_d = nc.dram_tensor("wu", [D, VSH], F16, kind="ExternalInput")
    logits_d = nc.dram_tensor("logits", [BS, VSH], F16, kind="ExternalOutput")

    # ------------- collective buffers -------------
    xb = nc.dram_tensor("xb", [D, T], F16)               # x_ln bounce
    xg = nc.dram_tensor("xg", [NCORES * D, T], F16, addr_space="Shared")
    zb = nc.dram_tensor("zb", [NCORES * 128, T], F16)    # z bounce (A2A in)
    zg = nc.dram_tensor("zg", [H * DH, T], F16)          # A2A out
    fb = nc.dram_tensor("fb", [D, T], F16)               # final bounce
    fg = nc.dram_tensor("fg", [NCORES * D, T], F16, addr_space="Shared")

    with tile.TileContext(nc) as tc:
        # PSUM pools live for the whole kernel (8 banks total).
        with tc.tile_pool(name="ps_mm", bufs=2, space="PSUM") as pps_mm, \
             tc.tile_pool(name="ps_sc", bufs=2, space="PSUM") as pps_sc, \
             tc.tile_pool(name="ps_z", bufs=2, space="PSUM") as pps_z, \
             tc.tile_pool(name="ps_vec", bufs=2, space="PSUM") as pps_vec:

            with ExitStack() as lctx:
                p1 = lctx.enter_context(tc.tile_pool(name="const", bufs=1))
                presid = lctx.enter_context(tc.tile_pool(name="presid",
                                                         bufs=8))
                pxln = lctx.enter_context(tc.tile_pool(name="pxln", bufs=8))
                pbig = lctx.enter_context(tc.tile_pool(name="pbig", bufs=3))
                pvaug = lctx.enter_context(tc.tile_pool(name="pvaug",
                                                        bufs=8))
                pxj = lctx.enter_context(tc.tile_pool(name="pxj", bufs=3))
                pvt = lctx.enter_context(tc.tile_pool(name="pvt", bufs=2))
                pex = lctx.enter_context(tc.tile_pool(name="pex", bufs=8))
                ppost = lctx.enter_context(tc.tile_pool(name="ppost",
                                                        bufs=18))
                pw = lctx.enter_context(tc.tile_pool(name="pw", bufs=4))
                pqw = lctx.enter_context(tc.tile_pool(name="pqw", bufs=3))
                psq = lctx.enter_context(tc.tile_pool(name="psq", bufs=2))
                ptmp = lctx.enter_context(tc.tile_pool(name="ptmp", bufs=2))
                pln = lctx.enter_context(tc.tile_pool(name="pln", bufs=8))
                prc = lctx.enter_context(tc.tile_pool(name="prc", bufs=3))

                # ---------- constants ----------
                ident = p1.tile([128, 128], F32, tag="ident")
                make_identity(nc, ident[:])
                identr = p1.tile([128, 128], F32R, tag="identr")
                nc.vector.tensor_copy(identr[:], ident[:])
                onesf = p1.tile([128, 128], F32, tag="onesf")
                nc.vector.memset(onesf[:], 1.0)
                ones16 = p1.tile([128, 8], F16, tag="ones16")
                nc.vector.memset(ones16[:], 1.0)
                ones_c = p1.tile([128, 1], F32R, tag="ones_c")
                nc.vector.tensor_copy(ones_c[:], onesf[:, 0:1])
                ones_r64 = p1.tile([1, 64], F32, tag="ones_r64")
                nc.vector.tensor_copy(ones_r64[:], onesf[0:1, 0:64])
                ones_r128 = p1.tile([1, 128], F32R, tag="ones_r128")
                nc.vector.tensor_copy(ones_r128[:], onesf[0:1, :])
                eps_t = p1.tile([1, 1], F32, tag="eps")
                nc.vector.memset(eps_t[:], EPS)
                # multiplicative causal masks for the 4 key tiles of a
                # diagonal 512x512 chunk; mask[k, q] = 1 iff q >= k + off
                masks = []
                for mi in range(4):
                    off = 128 * mi
                    mk = p1.tile([128, T], F16, tag=f"mask{mi}")
                    nc.gpsimd.memset(mk[:], 1.0)
                    nc.gpsimd.affine_select(
                        out=mk[:], in_=mk[:], compare_op=OP.is_ge,
                        fill=0.0, base=-off, pattern=[[1, T]],
                        channel_multiplier=-1)
                    masks.append(mk)

                # residual stream x^T, [D on partitions, T tokens], f32r
                resid = [presid.tile([128, T], F32R, tag="resid",
                                      name=f"resid{i}")
                         for i in range(DT)]

                def layer_norm(src_tiles, dst_tiles):
                    """dst = (src - mean_D) / sqrt(var_D + eps) per token;
                    x^T layout, stats over the partition (D) axis via
                    ones-matmuls.  rstd = exp(-0.5*ln(var+eps)) keeps the
                    ACT engine on the exp/ln table set (no sqrt set)."""
                    sum_ps = pps_vec.tile([1, T], F32, tag="vec")
                    sq_ps = pps_vec.tile([1, T], F32, tag="vec")
                    for d in range(DT):
                        sq = psq.tile([128, T], F32R, tag="sq")
                        nc.scalar.activation(sq[:], src_tiles[d][:],
                                             AF.Square)
                        nc.tensor.matmul(sum_ps[:], ones_c[:],
                                         src_tiles[d][:],
                                         start=(d == 0), stop=(d == DT - 1))
                        nc.tensor.matmul(sq_ps[:], ones_c[:], sq[:],
                                         start=(d == 0), stop=(d == DT - 1))
                    mean = pln.tile([1, T], F32R, tag="ln")
                    nc.scalar.mul(mean[:], sum_ps[:], 1.0 / D)
                    ems = pln.tile([1, T], F32, tag="ln")
                    nc.scalar.mul(ems[:], sq_ps[:], 1.0 / D)
                    m2 = pln.tile([1, T], F32, tag="ln")
                    nc.scalar.activation(m2[:], mean[:], AF.Square)
                    var = pln.tile([1, T], F32, tag="ln")
                    nc.vector.tensor_tensor(out=var[:], in0=ems[:],
                                            in1=m2[:], op=OP.subtract)
                    lnv = pln.tile([1, T], F32, tag="ln")
                    nc.scalar.activation(lnv[:], var[:], AF.Ln,
                                         bias=eps_t[:])
                    rstd = pln.tile([1, T], F32R, tag="ln")
                    nc.scalar.activation(rstd[:], lnv[:], AF.Exp,
                                         scale=-0.5)
                    bc_m = pps_vec.tile([128, T], F32, tag="vec")
                    nc.tensor.matmul(bc_m[:], ones_r128[:], mean[:],
                                     start=True, stop=True)
                    bc_r = pps_vec.tile([128, T], F32, tag="vec")
                    nc.tensor.matmul(bc_r[:], ones_r128[:], rstd[:],
                                     start=True, stop=True)
                    for d in range(DT):
                        tmp = ptmp.tile([128, T], F32, tag="lntmp")
                        nc.vector.tensor_tensor(out=tmp[:],
                                                in0=src_tiles[d][:],
                                                in1=bc_m[:], op=OP.subtract)
                        nc.vector.tensor_tensor(out=dst_tiles[d][:],
                                                in0=tmp[:], in1=bc_r[:],
                                                op=OP.mult)

                # ================= embedding =================
                with nc.named_scope("embed"), \
                     tc.tile_pool(name="pemb", bufs=8) as pemb, \
                     tc.tile_pool(name="pidx", bufs=2) as pidx:
                    wpos_sb = []
                    for d in range(DT):
                        wp = pemb.tile([128, T], F32, tag="wpos")
                        nc.sync.dma_start(wp[:], wpos_d[ts(d, 128), :])
                        wpos_sb.append(wp)
                    for t in range(T // 128):
                        it = pidx.tile([128, 1], I32, tag="idx")
                        nc.sync.dma_start(
                            it[:],
                            tok_d[ts(t, 128)].rearrange("(p o) -> p o", o=1))
                        xe = pidx.tile([128, D], F32, tag="xe")
                        nc.gpsimd.indirect_dma_start(
                            out=xe[:], out_offset=None, in_=we_d[:],
                            in_offset=bass.IndirectOffsetOnAxis(
                                ap=it[:, :1], axis=0))
                        for d in range(DT):
                            tp = pps_vec.tile([128, 128], F32, tag="vec")
                            nc.tensor.transpose(tp[:], xe[:, ts(d, 128)],
                                                ident[:])
                            nc.vector.tensor_tensor(
                                out=resid[d][:, ts(t, 128)], in0=tp[:],
                                in1=wpos_sb[d][:, ts(t, 128)], op=OP.add)

                # ================= layers =================
                for l in range(L):
                    # ---- LN1 + 8-core AllGather of x_ln^T ----
                    with nc.named_scope(f"l{l}_ln1"):
                        xln = [pxln.tile([128, T], F16, tag="xln",
                                         name=f"xln_{l}_{i}")
                               for i in range(DT)]
                        layer_norm(resid, xln)
                        for d in range(DT):
                            nc.sync.dma_start(xb[ts(d, 128), :], xln[d][:])
                        nc.gpsimd.collective_compute(
                            "AllGather", OP.bypass, replica_groups=ALL8,
                            ins=[xb[:]], outs=[xg[:]])

                    # ---- fused qkv + attention over 512-token chunks ----
                    with nc.named_scope(f"l{l}_attn"):
                        # per-core q/k in head-major layout
                        # [128 = 2 heads x 64dh, 4096 tokens]
                        qhp = pbig.tile([128, BS], F16, tag="big")
                        khp = pbig.tile([128, BS], F16, tag="big")
                        zT = pbig.tile([128, BS], F16, tag="big")
                        # v (normal layout) + ones column, per (head, batch):
                        # [128 tok, 8 keytiles, 65]
                        vaug = [[[pvaug.tile([128, 8, 65], F16, tag="vaug",
                                             name=f"vaug{l}_{h}_{b2}_{j}")
                                  for j in range(2)] for b2 in range(2)]
                                for h in range(2)]
                        for h in range(2):
                            for b2 in range(2):
                                for j in range(2):
                                    nc.vector.tensor_copy(
                                        vaug[h][b2][j][:, :, 64:65],
                                        ones16[:, :].rearrange(
                                            "p (a b) -> p a b", b=1))
                        wq = pqw.tile([128, 8, 128], F16, tag="qw")
                        nc.sync.dma_start(
                            wq[:],
                            wqk_d[l, :, 0:128].rearrange(
                                "(k p) c -> p k c", p=128))
                        wk = pqw.tile([128, 8, 128], F16, tag="qw")
                        nc.sync.dma_start(
                            wk[:],
                            wqk_d[l, :, 128:256].rearrange(
                                "(k p) c -> p k c", p=128))
                        wv = pqw.tile([128, 8, 128], F16, tag="qw")
                        nc.sync.dma_start(
                            wv[:],
                            wv_d[l].rearrange("(k p) c -> p k c", p=128))

                        for (b2, g) in CHUNKS:
                            c8 = 4 * b2 + g          # owning core index
                            col = slice(2048 * b2 + T * g,
                                        2048 * b2 + T * (g + 1))
                            xj = pxj.tile([128, 8, T], F16, tag="xj")
                            nc.sync.dma_start(
                                xj[:],
                                xg[ts(c8, D), :].rearrange(
                                    "(k p) c -> p k c", p=128))
                            # q / k for my 2 heads, this chunk
                            for w, dst in ((wq, qhp), (wk, khp)):
                                ps = pps_mm.tile([128, T], F32, tag="mm")
                                for k in range(DT):
                                    nc.tensor.matmul(
                                        ps[:], w[:, k, :], xj[:, k, :],
                                        start=(k == 0), stop=(k == DT - 1))
                                nc.vector.tensor_copy(dst[:, col], ps[:])
                            # v^T then PE-transpose to normal layout
                            psv = pps_mm.tile([128, T], F32, tag="mm")
                            for k in range(DT):
                                nc.tensor.matmul(
                                    psv[:], wv[:, k, :], xj[:, k, :],
                                    start=(k == 0), stop=(k == DT - 1))
                            vt = pvt.tile([128, T], F32R, tag="vt")
                            nc.vector.tensor_copy(vt[:], psv[:])
                            for tt in range(4):
                                kt = 4 * g + tt      # key tile in batch b2
                                tp = pps_vec.tile([128, 128], F32R,
                                                  tag="vec")
                                nc.tensor.transpose(
                                    tp[:], vt[:, ts(tt, 128)], identr[:])
                                for h in range(2):
                                    nc.vector.tensor_copy(
                                        vaug[h][b2][kt // 8][:, kt % 8,
                                                             0:64],
                                        tp[:, ts(h, 64)])
                            # ---- attention for this chunk's 512 queries
                            zps = [pps_z.tile([65, T], F32, tag="z",
                                              name=f"zps{hh}")
                                   for hh in range(2)]
                            nk = 4 * g + 4
                            for kt in range(nk):
                                for hh in range(2):
                                    scps = pps_sc.tile([128, T], F32,
                                                       tag="sc")
                                    nc.tensor.matmul(
                                        scps[:],
                                        khp[ts(hh, 64),
                                            2048 * b2 + 128 * kt:
                                            2048 * b2 + 128 * (kt + 1)],
                                        qhp[ts(hh, 64), col],
                                        start=True, stop=True)
                                    ex = pex.tile([128, T], F16, tag="ex")
                                    nc.scalar.activation(
                                        ex[:], scps[:], AF.Exp,
                                        scale=INV_SQRT_DH)
                                    if kt >= 4 * g:     # diagonal chunk
                                        nc.vector.tensor_tensor(
                                            out=ex[:], in0=ex[:],
                                            in1=masks[kt - 4 * g][:],
                                            op=OP.mult)
                                    nc.tensor.matmul(
                                        zps[hh][:],
                                        vaug[hh][b2][kt // 8][:, kt % 8,
                                                              0:65],
                                        ex[:], start=(kt == 0),
                                        stop=(kt == nk - 1))
                            for hh in range(2):
                                dn = prc.tile([1, T], F32, tag="dn")
                                nc.any.tensor_copy(dn[:], zps[hh][64:65, :])
                                rc = prc.tile([1, T], F32, tag="rc")
                                nc.vector.reciprocal_approx_fast(
                                    out=rc[:], in_=dn[:])
                                bc = pps_vec.tile([64, T], F32, tag="vec")
                                nc.tensor.matmul(
                                    bc[:], ones_r64[:], rc[:],
                                    start=True, stop=True)
                                bcs = prc.tile([64, T], F32, tag="bcs")
                                nc.any.tensor_copy(bcs[:], bc[:])
                                nc.vector.tensor_tensor(
                                    out=zT[ts(hh, 64), col],
                                    in0=zps[hh][0:64, :], in1=bcs[:],
                                    op=OP.mult)
                            # route this chunk's z to its owning core
                            nc.sync.dma_start(zb[ts(c8, 128), :],
                                              zT[:, col])

                    # ---- z AllToAll (head-shard -> token-shard) + W_O ----
                    with nc.named_scope(f"l{l}_wo"):
                        nc.gpsimd.collective_compute(
                            "AllToAll", OP.bypass, replica_groups=ALL8,
                            ins=[zb[:]], outs=[zg[:]])
                        zgt = pxj.tile([128, 8, T], F16, tag="xj")
                        nc.sync.dma_start(
                            zgt[:],
                            zg[:, :].rearrange("(k p) c -> p k c", p=128))
                        for m in range(DT):
                            w = pw.tile([128, 8, 128], F16, tag="w")
                            nc.sync.dma_start(
                                w[:],
                                wo_d[l, :, ts(m, 128)].rearrange(
                                    "(k p) c -> p k c", p=128))
                            ps = pps_mm.tile([128, T], F32, tag="mm")
                            for k in range(DT):
                                nc.tensor.matmul(
                                    ps[:], w[:, k, :], zgt[:, k, :],
                                    start=(k == 0), stop=(k == DT - 1))
                            nc.vector.tensor_tensor(out=resid[m][:],
                                                    in0=resid[m][:],
                                                    in1=ps[:], op=OP.add)

                    # ---- LN2 + MLP ----
                    with nc.named_scope(f"l{l}_mlp"):
                        xln2 = [pxln.tile([128, T], F16, tag="xln",
                                          name=f"xln2_{l}_{i}")
                                for i in range(DT)]
                        layer_norm(resid, xln2)
                        for half in range(2):
                            post = []
                            for mh in range(16):
                                m = 16 * half + mh
                                w = pw.tile([128, 8, 128], F16, tag="w")
                                nc.sync.dma_start(
                                    w[:],
                                    wi_d[l, :, ts(m, 128)].rearrange(
                                        "(k p) c -> p k c", p=128))
                                ps = pps_mm.tile([128, T], F32, tag="mm")
                                for k in range(DT):
                                    nc.tensor.matmul(
                                        ps[:], w[:, k, :], xln2[k][:],
                                        start=(k == 0), stop=(k == DT - 1))
                                po = ppost.tile([128, T], F16, tag="post")
                                nc.scalar.activation(po[:], ps[:],
                                                     AF.Gelu_apprx_tanh)
                                post.append(po)
                            for m in range(DT):
                                w = pw.tile([128, 16, 128], F16, tag="w")
                                nc.sync.dma_start(
                                    w[:],
                                    wout_d[l, ts(half, 2048),
                                           ts(m, 128)].rearrange(
                                        "(k p) c -> p k c", p=128))
                                ps = pps_mm.tile([128, T], F32, tag="mm")
                                for k in range(16):
                                    nc.tensor.matmul(
                                        ps[:], w[:, k, :], post[k][:],
                                        start=(k == 0), stop=(k == 15))
                                nc.vector.tensor_tensor(out=resid[m][:],
                                                        in0=resid[m][:],
                                                        in1=ps[:],
                                                        op=OP.add)

                # ---- final LN + 8-core gather ----
                with nc.named_scope("final_ln"):
                    xf = [pxln.tile([128, T], F16, tag="xln", name=f"xf{i}")
                          for i in range(DT)]
                    layer_norm(resid, xf)
                    for d in range(DT):
                        nc.sync.dma_start(fb[ts(d, 128), :], xf[d][:])
                    nc.gpsimd.collective_compute(
                        "AllGather", OP.bypass, replica_groups=ALL8,
                        ins=[fb[:]], outs=[fg[:]])

            # ================= unembed (vocab shard) =================
            with nc.named_scope("unembed"), \
                 tc.tile_pool(name="uf", bufs=32) as puf, \
                 tc.tile_pool(name="uw", bufs=2) as puw, \
                 tc.tile_pool(name="uo", bufs=3) as puo:
                ft = []
                for blk in range(NCORES):
                    for t in range(T // 128):
                        f = puf.tile([128, 8, 128], F16, tag="ft")
                        nc.sync.dma_start(
                            f[:],
                            fg[ts(blk, D), ts(t, 128)].rearrange(
                                "(k p) c -> p k c", p=128))
                        ft.append(f)
                for n in range(NV):
                    wn = min(512, VSH - 512 * n)      # 512 or 139
                    w = puw.tile([128, 8, 512], F16, tag="wu")
                    nc.sync.dma_start(
                        w[:, :, :wn],
                        wu_d[:, 512 * n: 512 * n + wn].rearrange(
                            "(k p) c -> p k c", p=128))
                    for tt in range(BS // 128):
                        if tt % 4 == 0:
                            o = puo.tile([128, 4, 512], F16, tag="lgo")
                        ps = pps_mm.tile([128, 512], F32, tag="mm")
                        for k in range(DT):
                            nc.tensor.matmul(ps[:, :wn], ft[tt][:, k, :],
                                             w[:, k, :wn],
                                             start=(k == 0),
                                             stop=(k == DT - 1))
                        nc.any.tensor_copy(o[:, tt % 4, :wn], ps[:, :wn])
                        if tt % 4 == 3:
                            nc.sync.dma_start(
                                logits_d[ts(tt // 4, 512),
                                         512 * n: 512 * n + wn].rearrange(
                                    "(a p) c -> p a c", p=128),
                                o[:, :, :wn])

    nc.compile()
    return nc


def _prep_inputs(inputs):
    """Validate + build the 8 per-core input maps (host-side sharding)."""
    inp = {k: np.asarray(v) for k, v in inputs.items()}

    for name in ('b_Q', 'b_K', 'b_V', 'b_O', 'b_in', 'b_out', 'b_U',
                 'ln1_b', 'ln2_b', 'lnf_b'):
        if inp[name].any():
            raise NotImplementedError(f"nonzero {name} not supported")
    for name in ('ln1_w', 'ln2_w', 'lnf_w'):
        if not np.all(inp[name] == 1.0):
            raise NotImplementedError(f"non-unit {name} not supported")

    tokens = inp['tokens'].astype(np.int32)                      # [B, S]
    W_E = np.ascontiguousarray(inp['W_E'], np.float32)           # [V, D]
    W_posT = np.ascontiguousarray(inp['W_pos'].T, np.float32)    # [D, S]
    WQ, WK, WV = inp['W_Q'], inp['W_K'], inp['W_V']              # [L,H,D,DH]
    WO = np.ascontiguousarray(
        inp['W_O'].reshape(L, H * DH, D), np.float16)            # [L,HDH,D]
    WI = np.ascontiguousarray(inp['W_in'], np.float16)           # [L,D,M]
    WOUT = np.ascontiguousarray(inp['W_out'], np.float16)        # [L,M,D]
    WU = inp['W_U'].astype(np.float32)                           # [D, V]

    in_maps = []
    for c in range(NCORES):
        b, g = c // G, c % G
        hsel = slice(2 * c, 2 * c + 2)
        # [L, 2, D, DH] -> [L, D, 2*DH]
        wq_c = WQ[:, hsel].transpose(0, 2, 1, 3).reshape(L, D, 2 * DH)
        wk_c = WK[:, hsel].transpose(0, 2, 1, 3).reshape(L, D, 2 * DH)
        wqk_c = np.ascontiguousarray(
            np.concatenate([wq_c, wk_c], axis=2), np.float16)
        wv_c = np.ascontiguousarray(
            WV[:, hsel].transpose(0, 2, 1, 3).reshape(L, D, 2 * DH),
            np.float16)
        lo, hi = VSH * c, min(VSH * (c + 1), V)
        wu_c = np.zeros((D, VSH), np.float16)
        wu_c[:, :hi - lo] = WU[:, lo:hi]
        in_maps.append({
            'tok': np.ascontiguousarray(tokens[b, ts(g, T)]),
            'we': W_E,
            'wposT': np.ascontiguousarray(W_posT[:, ts(g, T)]),
            'wqk': wqk_c,
            'wv': wv_c,
            'wo': WO,
            'wi': WI,
            'wout': WOUT,
            'wu': wu_c,
        })
    return in_maps


def kernel(**inputs):
    global _COMPILED
    if _COMPILED is None:
        _COMPILED = _build()
    nc = _COMPILED

    in_maps = _prep_inputs(inputs)
    trace = bool(int(os.environ.get('KERNEL_TRACE', '0')))
    res = run_bass_kernel_spmd(nc, in_maps, core_ids=list(range(NCORES)),
                               trace=trace)
    kernel.last_results = res

    logits = np.empty((B, S, V), np.float32)
    for c in range(NCORES):
        lg = res.results[c]['logits']                 # [4096, VSH] f16
        lo = VSH * c
        hi = min(VSH * (c + 1), V)
        if hi <= lo:
            continue
        for blk in range(NCORES):
            bb, gg = blk // G, blk % G
            logits[bb, ts(gg, T), lo:hi] = \
                lg[ts(blk, T), :hi - lo].astype(np.float32)
    return logits


# revision 12
# speedup vs baseline: 1.0654x; 1.0042x over previous
"""Trainium2 Bass kernel: 4-layer GPT-2-style transformer (B=2, S=2048, D=1024,
H=16, DH=64, M=4096, V=50257) on 8 NeuronCores.

Sharding (one SPMD program, no core-dependent control flow or addressing):
  - Residual stream / LN / MLP / W_O: sequence-parallel. Core c owns batch
    b = c//4, tokens [512*g, 512*(g+1)) with g = c%4.
  - Attention: head-parallel. Core c computes heads {2c, 2c+1} for BOTH
    batches and all tokens. Per layer: one 8-core AllGather of x_ln^T,
    then a fused qkv+attention pipeline over eight 512-token chunks
    (batch-interleaved for ILP), then one 8-core AllToAll routing z back
    from head-shards to token-shards.
  - Unembed: vocab-parallel. 8-core AllGather of final^T, then every core
    computes all 4096 tokens x its 6283-column vocab shard.
  - All matmul operands are float16 (weights converted host-side); PSUM
    accumulation is f32. The residual stream stays f32r on-chip.
"""

import sys, os
sys.path.insert(0, '/opt/trn_rl_repo')
os.environ.setdefault('MYCRO_LOCAL_CACHE', '1')

from contextlib import ExitStack

import numpy as np

import concourse.bass as bass
import concourse.bacc as bacc
import concourse.mybir as mybir
import concourse.tile as tile
from concourse.bass_utils import run_bass_kernel_spmd
from concourse.masks import make_identity

# model dims
B, S, V, D, H, DH, MLPD, L = 2, 2048, 50257, 1024, 16, 64, 4096, 4
EPS = 1e-5
NCORES = 8
G = 4                 # sequence-parallel degree within a batch
T = S // G            # 512 local tokens per core
BS = B * S            # 4096 total tokens
DT = D // 128         # 8 d-tiles
INV_SQRT_DH = float(1.0 / np.sqrt(DH))
VSH = (V + NCORES - 1) // NCORES      # 6283 vocab shard width
NV = (VSH + 511) // 512               # 13 vocab n-tiles (last is 139 wide)

F32 = mybir.dt.float32
F32R = mybir.dt.float32r
I32 = mybir.dt.int32
F16 = mybir.dt.float16
AF = mybir.ActivationFunctionType
OP = mybir.AluOpType

ALL8 = [[0, 1, 2, 3, 4, 5, 6, 7]]

# chunk processing order: (b2, g) pairs, batch-interleaved so the two
# batches' attention chains overlap
CHUNKS = [(0, 0), (1, 0), (0, 1), (1, 1), (0, 2), (1, 2), (0, 3), (1, 3)]

_COMPILED = None


def ts(i, n):
    return slice(i * n, (i + 1) * n)


def _build():
    nc = bacc.Bacc("TRN2", target_bir_lowering=False, debug=False,
                   num_devices=NCORES)

    # ---------------- I/O -----------------
    tok_d = nc.dram_tensor("tok", [T], I32, kind="ExternalInput")
    we_d = nc.dram_tensor("we", [V, D], F32, kind="ExternalInput")
    wpos_d = nc.dram_tensor("wposT", [D, T], F32, kind="ExternalInput")
    # per-core head slice: q-pair (128) | k-pair (128)
    wqk_d = nc.dram_tensor("wqk", [L, D, 256], F16, kind="ExternalInput")
    wv_d = nc.dram_tensor("wv", [L, D, 128], F16, kind="ExternalInput")
    wo_d = nc.dram_tensor("wo", [L, H * DH, D], F16, kind="ExternalInput")
    wi_d = nc.dram_tensor("wi", [L, D, MLPD], F16, kind="ExternalInput")
    wout_d = nc.dram_tensor("wout", [L, MLPD, D], F16, kind="ExternalInput")
    wu_d = nc.dram_tensor("wu", [D, VSH], F16, kind="ExternalInput")
    logits_d = nc.dram_tensor("logits", [BS, VSH], F16, kind="ExternalOutput")

    # ------------- collective buffers -------------
    xb = nc.dram_tensor("xb", [D, T], F16)               # x_ln bounce
    xg = nc.dram_tensor("xg", [NCORES * D, T], F16, addr_space="Shared")
    zb = nc.dram_tensor("zb", [NCORES * 128, T], F16)    # z bounce (A2A in)
    zg = nc.dram_tensor("zg", [H * DH, T], F16)          # A2A out
    fb = nc.dram_tensor("fb", [D, T], F16)               # final bounce
    fg = nc.dram_tensor("fg", [NCORES * D, T], F16, addr_space="Shared")

    with tile.TileContext(nc) as tc:
        # PSUM pools live for the whole kernel (8 banks total).
        with tc.tile_pool(name="ps_mm", bufs=2, space="PSUM") as pps_mm, \
             tc.tile_pool(name="ps_sc", bufs=2, space="PSUM") as pps_sc, \
             tc.tile_pool(name="ps_z", bufs=2, space="PSUM") as pps_z, \
             tc.tile_pool(name="ps_vec", bufs=2, space="PSUM") as pps_vec:

            with ExitStack() as lctx:
                p1 = lctx.enter_context(tc.tile_pool(name="const", bufs=1))
                presid = lctx.enter_context(tc.tile_pool(name="presid",
                                                         bufs=8))
                pxln = lctx.enter_context(tc.tile_pool(name="pxln", bufs=8))
                pbig = lctx.enter_context(tc.tile_pool(name="pbig", bufs=3))
                pvaug = lctx.enter_context(tc.tile_pool(name="pvaug",
                                                        bufs=8))
                pxj = lctx.enter_context(tc.tile_pool(name="pxj", bufs=3))
                pvt = lctx.enter_context(tc.tile_pool(name="pvt", bufs=2))
                pex = lctx.enter_context(tc.tile_pool(name="pex", bufs=8))
                ppost = lctx.enter_context(tc.tile_pool(name="ppost",
                                                        bufs=18))
                pw = lctx.enter_context(tc.tile_pool(name="pw", bufs=4))
                pqw = lctx.enter_context(tc.tile_pool(name="pqw", bufs=3))
                psq = lctx.enter_context(tc.tile_pool(name="psq", bufs=2))
                ptmp = lctx.enter_context(tc.tile_pool(name="ptmp", bufs=2))
                pln = lctx.enter_context(tc.tile_pool(name="pln", bufs=8))
                prc = lctx.enter_context(tc.tile_pool(name="prc", bufs=3))

                # ---------- constants ----------
                ident = p1.tile([128, 128], F32, tag="ident")
                make_identity(nc, ident[:])
                identr = p1.tile([128, 128], F32R, tag="identr")
                nc.vector.tensor_copy(identr[:], ident[:])
                onesf = p1.tile([128, 128], F32, tag="onesf")
                nc.vector.memset(onesf[:], 1.0)
                ones16 = p1.tile([128, 8], F16, tag="ones16")
                nc.vector.memset(ones16[:], 1.0)
                ones_c = p1.tile([128, 1], F32R, tag="ones_c")
                nc.vector.tensor_copy(ones_c[:], onesf[:, 0:1])
                ones_r64 = p1.tile([1, 64], F32, tag="ones_r64")
                nc.vector.tensor_copy(ones_r64[:], onesf[0:1, 0:64])
                ones_r128 = p1.tile([1, 128], F32R, tag="ones_r128")
                nc.vector.tensor_copy(ones_r128[:], onesf[0:1, :])
                eps_t = p1.tile([1, 1], F32, tag="eps")
                nc.vector.memset(eps_t[:], EPS)
                # multiplicative causal masks for the 4 key tiles of a
                # diagonal 512x512 chunk; mask[k, q] = 1 iff q >= k + off
                masks = []
                for mi in range(4):
                    off = 128 * mi
                    mk = p1.tile([128, T], F16, tag=f"mask{mi}")
                    nc.gpsimd.memset(mk[:], 1.0)
                    nc.gpsimd.affine_select(
                        out=mk[:], in_=mk[:], compare_op=OP.is_ge,
                        fill=0.0, base=-off, pattern=[[1, T]],
                        channel_multiplier=-1)
                    masks.append(mk)

                # residual stream x^T, [D on partitions, T tokens], f32r
                resid = [presid.tile([128, T], F32R, tag="resid",
                                      name=f"resid{i}")
                         for i in range(DT)]

                def layer_norm(src_tiles, dst_tiles):
                    """dst = (src - mean_D) / sqrt(var_D + eps) per token;
                    x^T layout, stats over the partition (D) axis via
                    ones-matmuls.  rstd = exp(-0.5*ln(var+eps)) keeps the
                    ACT engine on the exp/ln table set (no sqrt set)."""
                    sum_ps = pps_vec.tile([1, T], F32, tag="vec")
                    sq_ps = pps_vec.tile([1, T], F32, tag="vec")
                    for d in range(DT):
                        sq = psq.tile([128, T], F32R, tag="sq")
                        nc.scalar.activation(sq[:], src_tiles[d][:],
                                             AF.Square)
                        nc.tensor.matmul(sum_ps[:], ones_c[:],
                                         src_tiles[d][:],
                                         start=(d == 0), stop=(d == DT - 1))
                        nc.tensor.matmul(sq_ps[:], ones_c[:], sq[:],
                                         start=(d == 0), stop=(d == DT - 1))
                    mean = pln.tile([1, T], F32R, tag="ln")
                    nc.scalar.mul(mean[:], sum_ps[:], 1.0 / D)
                    ems = pln.tile([1, T], F32, tag="ln")
                    nc.scalar.mul(ems[:], sq_ps[:], 1.0 / D)
                    m2 = pln.tile([1, T], F32, tag="ln")
                    nc.scalar.activation(m2[:], mean[:], AF.Square)
                    var = pln.tile([1, T], F32, tag="ln")
                    nc.vector.tensor_tensor(out=var[:], in0=ems[:],
                                            in1=m2[:], op=OP.subtract)
                    lnv = pln.tile([1, T], F32, tag="ln")
                    nc.scalar.activation(lnv[:], var[:], AF.Ln,
                                         bias=eps_t[:])
                    rstd = pln.tile([1, T], F32R, tag="ln")
                    nc.scalar.activation(rstd[:], lnv[:], AF.Exp,
                                         scale=-0.5)
                    bc_m = pps_vec.tile([128, T], F32, tag="vec")
                    nc.tensor.matmul(bc_m[:], ones_r128[:], mean[:],
                                     start=True, stop=True)
                    bc_r = pps_vec.tile([128, T], F32, tag="vec")
                    nc.tensor.matmul(bc_r[:], ones_r128[:], rstd[:],
                                     start=True, stop=True)
                    for d in range(DT):
                        tmp = ptmp.tile([128, T], F32, tag="lntmp")
                        nc.vector.tensor_tensor(out=tmp[:],
                                                in0=src_tiles[d][:],
                                                in1=bc_m[:], op=OP.subtract)
                        nc.vector.tensor_tensor(out=dst_tiles[d][:],
                                                in0=tmp[:], in1=bc_r[:],
                                                op=OP.mult)

                # ================= embedding =================
                with nc.named_scope("embed"), \
                     tc.tile_pool(name="pemb", bufs=8) as pemb, \
                     tc.tile_pool(name="pidx", bufs=2) as pidx:
                    wpos_sb = []
                    for d in range(DT):
                        wp = pemb.tile([128, T], F32, tag="wpos")
                        nc.sync.dma_start(wp[:], wpos_d[ts(d, 128), :])
                        wpos_sb.append(wp)
                    for t in range(T // 128):
                        it = pidx.tile([128, 1], I32, tag="idx")
                        nc.sync.dma_start(
                            it[:],
                            tok_d[ts(t, 128)].rearrange("(p o) -> p o", o=1))
                        xe = pidx.tile([128, D], F32, tag="xe")
                        nc.gpsimd.indirect_dma_start(
                            out=xe[:], out_offset=None, in_=we_d[:],
                            in_offset=bass.IndirectOffsetOnAxis(
                                ap=it[:, :1], axis=0))
                        for d in range(DT):
                            tp = pps_vec.tile([128, 128], F32, tag="vec")
                            nc.tensor.transpose(tp[:], xe[:, ts(d, 128)],
                                                ident[:])
                            nc.vector.tensor_tensor(
                                out=resid[d][:, ts(t, 128)], in0=tp[:],
                                in1=wpos_sb[d][:, ts(t, 128)], op=OP.add)

                # ================= layers =================
                for l in range(L):
                    # ---- LN1 + 8-core AllGather of x_ln^T ----
                    with nc.named_scope(f"l{l}_ln1"):
                        xln = [pxln.tile([128, T], F16, tag="xln",
                                         name=f"xln_{l}_{i}")
                               for i in range(DT)]
                        layer_norm(resid, xln)
                        for d in range(DT):
                            nc.sync.dma_start(xb[ts(d, 128), :], xln[d][:])
                        nc.gpsimd.collective_compute(
                            "AllGather", OP.bypass, replica_groups=ALL8,
                            ins=[xb[:]], outs=[xg[:]])

                    # ---- fused qkv + attention over 512-token chunks ----
                    with nc.named_scope(f"l{l}_attn"):
                        # per-core q/k in head-major layout
                        # [128 = 2 heads x 64dh, 4096 tokens]
                        qhp = pbig.tile([128, BS], F16, tag="big")
                        khp = pbig.tile([128, BS], F16, tag="big")
                        zT = pbig.tile([128, BS], F16, tag="big")
                        # v (normal layout) + ones column, per (head, batch):
                        # [128 tok, 8 keytiles, 65]
                        vaug = [[[pvaug.tile([128, 8, 65], F16, tag="vaug",
                                             name=f"vaug{l}_{h}_{b2}_{j}")
                                  for j in range(2)] for b2 in range(2)]
                                for h in range(2)]
                        for h in range(2):
                            for b2 in range(2):
                                for j in range(2):
                                    nc.vector.tensor_copy(
                                        vaug[h][b2][j][:, :, 64:65],
                                        ones16[:, :].rearrange(
                                            "p (a b) -> p a b", b=1))
                        wq = pqw.tile([128, 8, 128], F16, tag="qw")
                        nc.sync.dma_start(
                            wq[:],
                            wqk_d[l, :, 0:128].rearrange(
                                "(k p) c -> p k c", p=128))
                        wk = pqw.tile([128, 8, 128], F16, tag="qw")
                        nc.sync.dma_start(
                            wk[:],
                            wqk_d[l, :, 128:256].rearrange(
                                "(k p) c -> p k c", p=128))
                        wv = pqw.tile([128, 8, 128], F16, tag="qw")
                        nc.sync.dma_start(
                            wv[:],
                            wv_d[l].rearrange("(k p) c -> p k c", p=128))

                        for (b2, g) in CHUNKS:
                            c8 = 4 * b2 + g          # owning core index
                            col = slice(2048 * b2 + T * g,
                                        2048 * b2 + T * (g + 1))
                            xj = pxj.tile([128, 8, T], F16, tag="xj")
                            nc.sync.dma_start(
                                xj[:],
                                xg[ts(c8, D), :].rearrange(
                                    "(k p) c -> p k c", p=128))
                            # q / k for my 2 heads, this chunk
                            for w, dst in ((wq, qhp), (wk, khp)):
                                ps = pps_mm.tile([128, T], F32, tag="mm")
                                for k in range(DT):
                                    nc.tensor.matmul(
                                        ps[:], w[:, k, :], xj[:, k, :],
                                        start=(k == 0), stop=(k == DT - 1))
                                nc.vector.tensor_copy(dst[:, col], ps[:])
                            # v^T then PE-transpose to normal layout
                            psv = pps_mm.tile([128, T], F32, tag="mm")
                            for k in range(DT):
                                nc.tensor.matmul(
                                    psv[:], wv[:, k, :], xj[:, k, :],
                                    start=(k == 0), stop=(k == DT - 1))
                            vt = pvt.tile([128, T], F32R, tag="vt")
                            nc.vector.tensor_copy(vt[:], psv[:])
                            for tt in range(4):
                                kt = 4 * g + tt      # key tile in batch b2
                                tp = pps_vec.tile([128, 128], F32R,
                                                  tag="vec")
                                nc.tensor.transpose(
                                    tp[:], vt[:, ts(tt, 128)], identr[:])
                                for h in range(2):
                                    nc.vector.tensor_copy(
                                        vaug[h][b2][kt // 8][:, kt % 8,
                                                             0:64],
                                        tp[:, ts(h, 64)])
                            # ---- attention for this chunk's 512 queries
                            zps = [pps_z.tile([65, T], F32, tag="z",
                                              name=f"zps{hh}")
                                   for hh in range(2)]
                            nk = 4 * g + 4
                            for kt in range(nk):
                                for hh in range(2):
                                    scps = pps_sc.tile([128, T], F32,
                                                       tag="sc")
                                    nc.tensor.matmul(
                                        scps[:],
                                        khp[ts(hh, 64),
                                            2048 * b2 + 128 * kt:
                                            2048 * b2 + 128 * (kt + 1)],
                                        qhp[ts(hh, 64), col],
                                        start=True, stop=True)
                                    ex = pex.tile([128, T], F16, tag="ex")
                                    nc.scalar.activation(
                                        ex[:], scps[:], AF.Exp,
                                        scale=INV_SQRT_DH)
                                    if kt >= 4 * g:     # diagonal chunk
                                        nc.vector.tensor_tensor(
                                            out=ex[:], in0=ex[:],
                                            in1=masks[kt - 4 * g][:],
                                            op=OP.mult)
                                    nc.tensor.matmul(
                                        zps[hh][:],
                                        vaug[hh][b2][kt // 8][:, kt % 8,
                                                              0:65],
                                        ex[:], start=(kt == 0),
                                        stop=(kt == nk - 1))
                            for hh in range(2):
                                dn = prc.tile([1, T], F32, tag="dn")
                                nc.any.tensor_copy(dn[:], zps[hh][64:65, :])
                                rc = prc.tile([1, T], F32, tag="rc")
                                nc.vector.reciprocal_approx_fast(
                                    out=rc[:], in_=dn[:])
                                bc = pps_vec.tile([64, T], F32, tag="vec")
                                nc.tensor.matmul(
                                    bc[:], ones_r64[:], rc[:],
                                    start=True, stop=True)
                                bcs = prc.tile([64, T], F32, tag="bcs")
                                nc.any.tensor_copy(bcs[:], bc[:])
                                nc.vector.tensor_tensor(
                                    out=zT[ts(hh, 64), col],
                                    in0=zps[hh][0:64, :], in1=bcs[:],
                                    op=OP.mult)
                            # route this chunk's z to its owning core
                            nc.sync.dma_start(zb[ts(c8, 128), :],
                                              zT[:, col])

                    # ---- z AllToAll (head-shard -> token-shard) + W_O ----
                    with nc.named_scope(f"l{l}_wo"):
                        nc.gpsimd.collective_compute(
                            "AllToAll", OP.bypass, replica_groups=ALL8,
                            ins=[zb[:]], outs=[zg[:]])
                        zgt = pxj.tile([128, 8, T], F16, tag="xj")
                        nc.sync.dma_start(
                            zgt[:],
                            zg[:, :].rearrange("(k p) c -> p k c", p=128))
                        for m in range(DT):
                            w = pw.tile([128, 8, 128], F16, tag="w")
                            nc.sync.dma_start(
                                w[:],
                                wo_d[l, :, ts(m, 128)].rearrange(
                                    "(k p) c -> p k c", p=128))
                            pool_ = pps_mm if m % 2 == 0 else pps_sc
                            ps = pool_.tile([128, T], F32,
                                            tag="mm" if m % 2 == 0 else "sc")
                            for k in range(DT):
                                nc.tensor.matmul(
                                    ps[:], w[:, k, :], zgt[:, k, :],
                                    start=(k == 0), stop=(k == DT - 1))
                            nc.vector.tensor_tensor(out=resid[m][:],
                                                    in0=resid[m][:],
                                                    in1=ps[:], op=OP.add)

                    # ---- LN2 + MLP ----
                    with nc.named_scope(f"l{l}_mlp"):
                        xln2 = [pxln.tile([128, T], F16, tag="xln",
                                          name=f"xln2_{l}_{i}")
                                for i in range(DT)]
                        layer_norm(resid, xln2)
                        for half in range(2):
                            post = []
                            for mh in range(16):
                                m = 16 * half + mh
                                w = pw.tile([128, 8, 128], F16, tag="w")
                                nc.sync.dma_start(
                                    w[:],
                                    wi_d[l, :, ts(m, 128)].rearrange(
                                        "(k p) c -> p k c", p=128))
                                pool_ = pps_mm if mh % 2 == 0 else pps_sc
                                ps = pool_.tile(
                                    [128, T], F32,
                                    tag="mm" if mh % 2 == 0 else "sc")
                                for k in range(DT):
                                    nc.tensor.matmul(
                                        ps[:], w[:, k, :], xln2[k][:],
                                        start=(k == 0), stop=(k == DT - 1))
                                po = ppost.tile([128, T], F16, tag="post")
                                nc.scalar.activation(po[:], ps[:],
                                                     AF.Gelu_apprx_tanh)
                                post.append(po)
                            for m in range(DT):
                                w = pw.tile([128, 16, 128], F16, tag="w")
                                nc.sync.dma_start(
                                    w[:],
                                    wout_d[l, ts(half, 2048),
                                           ts(m, 128)].rearrange(
                                        "(k p) c -> p k c", p=128))
                                pool_ = pps_mm if m % 2 == 0 else pps_sc
                                ps = pool_.tile(
                                    [128, T], F32,
                                    tag="mm" if m % 2 == 0 else "sc")
                                for k in range(16):
                                    nc.tensor.matmul(
                                        ps[:], w[:, k, :], post[k][:],
                                        start=(k == 0), stop=(k == 15))
                                nc.vector.tensor_tensor(out=resid[m][:],
                                                        in0=resid[m][:],
                                                        in1=ps[:],
                                                        op=OP.add)

                # ---- final LN + 8-core gather ----
                with nc.named_scope("final_ln"):
                    xf = [pxln.tile([128, T], F16, tag="xln", name=f"xf{i}")
                          for i in range(DT)]
                    layer_norm(resid, xf)
                    for d in range(DT):
                        nc.sync.dma_start(fb[ts(d, 128), :], xf[d][:])
                    nc.gpsimd.collective_compute(
                        "AllGather", OP.bypass, replica_groups=ALL8,
                        ins=[fb[:]], outs=[fg[:]])

            # ================= unembed (vocab shard) =================
            with nc.named_scope("unembed"), \
                 tc.tile_pool(name="uf", bufs=32) as puf, \
                 tc.tile_pool(name="uw", bufs=2) as puw, \
                 tc.tile_pool(name="uo", bufs=3) as puo:
                ft = []
                for blk in range(NCORES):
                    for t in range(T // 128):
                        f = puf.tile([128, 8, 128], F16, tag="ft")
                        nc.sync.dma_start(
                            f[:],
                            fg[ts(blk, D), ts(t, 128)].rearrange(
                                "(k p) c -> p k c", p=128))
                        ft.append(f)
                for n in range(NV):
                    wn = min(512, VSH - 512 * n)      # 512 or 139
                    w = puw.tile([128, 8, 512], F16, tag="wu")
                    nc.sync.dma_start(
                        w[:, :, :wn],
                        wu_d[:, 512 * n: 512 * n + wn].rearrange(
                            "(k p) c -> p k c", p=128))
                    for tt in range(BS // 128):
                        if tt % 4 == 0:
                            o = puo.tile([128, 4, 512], F16, tag="lgo")
                        pool_ = pps_mm if tt % 2 == 0 else pps_sc
                        ps = pool_.tile([128, 512], F32,
                                        tag="mm" if tt % 2 == 0 else "sc")
                        for k in range(DT):
                            nc.tensor.matmul(ps[:, :wn], ft[tt][:, k, :],
                                             w[:, k, :wn],
                                             start=(k == 0),
                                             stop=(k == DT - 1))
                        nc.any.tensor_copy(o[:, tt % 4, :wn], ps[:, :wn])
                        if tt % 4 == 3:
                            nc.sync.dma_start(
                                logits_d[ts(tt // 4, 512),
                                         512 * n: 512 * n + wn].rearrange(
                                    "(a p) c -> p a c", p=128),
                                o[:, :, :wn])

    nc.compile()
    return nc


def _prep_inputs(inputs):
    """Validate + build the 8 per-core input maps (host-side sharding)."""
    inp = {k: np.asarray(v) for k, v in inputs.items()}

    for name in ('b_Q', 'b_K', 'b_V', 'b_O', 'b_in', 'b_out', 'b_U',
                 'ln1_b', 'ln2_b', 'lnf_b'):
        if inp[name].any():
            raise NotImplementedError(f"nonzero {name} not supported")
    for name in ('ln1_w', 'ln2_w', 'lnf_w'):
        if not np.all(inp[name] == 1.0):
            raise NotImplementedError(f"non-unit {name} not supported")

    tokens = inp['tokens'].astype(np.int32)                      # [B, S]
    W_E = np.ascontiguousarray(inp['W_E'], np.float32)           # [V, D]
    W_posT = np.ascontiguousarray(inp['W_pos'].T, np.float32)    # [D, S]
    WQ, WK, WV = inp['W_Q'], inp['W_K'], inp['W_V']              # [L,H,D,DH]
    WO = np.ascontiguousarray(
        inp['W_O'].reshape(L, H * DH, D), np.float16)            # [L,HDH,D]
    WI = np.ascontiguousarray(inp['W_in'], np.float16)           # [L,D,M]
    WOUT = np.ascontiguousarray(inp['W_out'], np.float16)        # [L,M,D]
    WU = inp['W_U'].astype(np.float32)                           # [D, V]

    in_maps = []
    for c in range(NCORES):
        b, g = c // G, c % G
        hsel = slice(2 * c, 2 * c + 2)
        # [L, 2, D, DH] -> [L, D, 2*DH]
        wq_c = WQ[:, hsel].transpose(0, 2, 1, 3).reshape(L, D, 2 * DH)
        wk_c = WK[:, hsel].transpose(0, 2, 1, 3).reshape(L, D, 2 * DH)
        wqk_c = np.ascontiguousarray(
            np.concatenate([wq_c, wk_c], axis=2), np.float16)
        wv_c = np.ascontiguousarray(
            WV[:, hsel].transpose(0, 2, 1, 3).reshape(L, D, 2 * DH),
            np.float16)
        lo, hi = VSH * c, min(VSH * (c + 1), V)
        wu_c = np.zeros((D, VSH), np.float16)
        wu_c[:, :hi - lo] = WU[:, lo:hi]
        in_maps.append({
            'tok': np.ascontiguousarray(tokens[b, ts(g, T)]),
            'we': W_E,
            'wposT': np.ascontiguousarray(W_posT[:, ts(g, T)]),
            'wqk': wqk_c,
            'wv': wv_c,
            'wo': WO,
            'wi': WI,
            'wout': WOUT,
            'wu': wu_c,
        })
    return in_maps


def kernel(**inputs):
    global _COMPILED
    if _COMPILED is None:
        _COMPILED = _build()
    nc = _COMPILED

    in_maps = _prep_inputs(inputs)
    trace = bool(int(os.environ.get('KERNEL_TRACE', '0')))
    res = run_bass_kernel_spmd(nc, in_maps, core_ids=list(range(NCORES)),
                               trace=trace)
    kernel.last_results = res

    logits = np.empty((B, S, V), np.float32)
    for c in range(NCORES):
        lg = res.results[c]['logits']                 # [4096, VSH] f16
        lo = VSH * c
        hi = min(VSH * (c + 1), V)
        if hi <= lo:
            continue
        for blk in range(NCORES):
            bb, gg = blk // G, blk % G
            logits[bb, ts(gg, T), lo:hi] = \
                lg[ts(blk, T), :hi - lo].astype(np.float32)
    return logits
